# revision 43
# baseline (speedup 1.0000x reference)
"""Distributed GQA attention kernel for 8 TRN2 NeuronCores.

Sharding (tensor-parallel over heads): core i owns q-heads [8i, 8i+8) and
kv-head i (GQA n_rep=8, so one kv head serves all 8 local q heads). Each core:
  1. QKV projection from the full x (weights pre-transposed host-side),
     computed in f32r (full-rate fp32-rounded matmuls).
  2. RoPE on qT/kT in [d, s] layout (sin staged sign-folded).
  3. Causal attention per head in transposed-score layout [k, q]:
     exp(scale*s) with no max subtraction (scores are O(6)), the attention
     sink enters as +exp(sink) in the denominator, and denominators ride an
     extra ones-column appended to v.
  4. Local slice of the output projection -> partial yT [2880, 1536].
  5. ReduceScatter(add) over the 8 cores; core i gets yT rows [360i, 360i+360),
     adds the wo bias slice. Host concatenates/transposes shards.
"""

import contextlib
import ctypes
import os
import sys

import numpy as np

sys.path.insert(0, "/opt/trn_rl_repo")

S = 1536
HID = 2880
D = 64
HL = 8          # local q heads per core
CORES = 8
SCQ = 512       # QKV moving chunk
NSC = S // SCQ
QC = 512        # attention q chunk
NQC = S // QC
KBN = S // 128  # 12 k blocks
VA = 68         # v_aug padded width (f32r moving dim must be 4-aligned)
CBF = 22        # full 128-row contraction blocks (2880 = 22*128 + 64)
MBN = 23        # wo output row blocks (22 full + one 64)
JBN = 4         # 512 local j rows = 4 blocks

_EXEC_TIME_NS = [None]


def _install_hooks():
    import types

    import antenv

    try:
        from antenv import axon_hooks
    except ImportError:
        axon_hooks = types.ModuleType("antenv.axon_hooks")
        _holder = {"hook": None}
        axon_hooks.set_axon_ntff_profile_hook = lambda h: _holder.update(hook=h)
        axon_hooks.get_axon_ntff_profile_hook = lambda: _holder["hook"]
        sys.modules["antenv.axon_hooks"] = axon_hooks
        antenv.axon_hooks = axon_hooks

    so_path = "/opt/axon/libaxon_pjrt.so"
    hook = None
    if os.path.exists(so_path):
        lib = ctypes.CDLL(so_path)
        if hasattr(lib, "axon_start_nrt_profile"):
            lib.axon_start_nrt_profile.argtypes = [
                ctypes.POINTER(ctypes.c_int64),
                ctypes.c_size_t,
            ]
            lib.axon_start_nrt_profile.restype = ctypes.c_int64
            lib.axon_stop_nrt_profile.argtypes = [ctypes.c_char_p]
            lib.axon_stop_nrt_profile.restype = ctypes.c_int64

            @contextlib.contextmanager
            def hook(output_dir, device_ids):
                import jax

                jax.devices()
                if device_ids:
                    ids = (ctypes.c_int64 * len(device_ids))(*device_ids)
                    rc = lib.axon_start_nrt_profile(ids, len(device_ids))
                else:
                    rc = lib.axon_start_nrt_profile(None, 0)
                if rc != 0:
                    raise RuntimeError(f"axon_start_nrt_profile rc={rc}")
                try:
                    yield
                finally:
                    n = lib.axon_stop_nrt_profile(str(output_dir).encode())
                    print(f"profile: {n} file(s) written to {output_dir}")

    axon_hooks.set_axon_ntff_profile_hook(hook)

    import concourse.bass_utils as bu

    bu.upload_artifacts = lambda tmpdir: f"local://{tmpdir}"

    if os.environ.get("BASS_LDW_OPT", "0") == "1" and not getattr(
        bu, "_ldw_patched", False
    ):
        _orig_run = bu.run_command

        def _run(cmd, *a, **k):
            cmd = [
                c.replace("--enable-ldw-opt=false", "--enable-ldw-opt=true")
                if isinstance(c, str)
                else c
                for c in cmd
            ]
            return _orig_run(cmd, *a, **k)

        bu.run_command = _run
        bu._ldw_patched = True


def build_graph():
    import concourse.mybir as mybir
    import concourse.tile as tile
    from concourse import bacc
    from concourse.masks import make_identity

    F32 = mybir.dt.float32
    BF16 = mybir.dt.bfloat16

    nc = bacc.Bacc("TRN2", target_bir_lowering=False, debug=False, num_devices=CORES)

    xT = nc.declare_dram_parameter("xT", [HID, S], BF16, isOutput=False)
    wT = nc.declare_dram_parameter("wT", [HID, 640], BF16, isOutput=False)
    bq = nc.declare_dram_parameter("bq", [64, HL], F32, isOutput=False)
    bk = nc.declare_dram_parameter("bk", [64, 1], F32, isOutput=False)
    bv = nc.declare_dram_parameter("bv", [64, 1], F32, isOutput=False)
    cosT = nc.declare_dram_parameter("cosT", [64, S], F32, isOutput=False)
    sinTs = nc.declare_dram_parameter("sinTs", [64, S], F32, isOutput=False)
    woT = nc.declare_dram_parameter("woT", [512, HID], BF16, isOutput=False)
    wob8 = nc.declare_dram_parameter("wob8", [128, MBN], F32, isOutput=False)
    esink = nc.declare_dram_parameter("esink", [128, HL], F32, isOutput=False)
    out = nc.declare_dram_parameter("out", [360, S], F32, isOutput=True)

    # qc processing order [2, 1, 0]; the last-processed chunk (qc=0) gets its
    # wo/ReduceScatter split into column halves so the serial RS tail is small.
    QC_ORDER = [2, 1, 0]
    # (qc, col_off_within_qc, width) wo/RS segments, in issue order
    WO_SEGS = {2: [(0, QC)], 1: [(0, QC)], 0: [(0, QC)]}
    yT_part = {}
    yT_red = {}
    for qc in range(NQC):
        for off, w in WO_SEGS[qc]:
            yT_part[(qc, off)] = nc.dram_tensor(f"yT_part_{qc}_{off}", [HID, w], BF16)
            yT_red[(qc, off)] = nc.dram_tensor(f"yT_red_{qc}_{off}", [360, w], BF16)

    Exp = mybir.ActivationFunctionType.Exp

    # QKV contraction pieces: groups of 128-row c-blocks (22 full + one 64-row)
    PIECES = [(0, 4), (4, 4), (8, 4), (12, 4), (16, 4), (20, 2)]

    with tile.TileContext(nc) as tc:
        with contextlib.ExitStack() as stack:
            consts = stack.enter_context(tc.tile_pool(name="consts", bufs=1))
            qkvout = stack.enter_context(tc.tile_pool(name="qkvout", bufs=1))
            small = stack.enter_context(tc.tile_pool(name="small", bufs=6))
            ytp = stack.enter_context(tc.tile_pool(name="ytp", bufs=4))

            bqt = consts.tile([64, HL], F32, tag="bq")
            bkt = consts.tile([64, 1], F32, tag="bk")
            bvt = consts.tile([64, 1], F32, tag="bv")
            cost = consts.tile([64, S], F32, tag="cos")
            sint = consts.tile([64, S], F32, tag="sin")
            wob8t = consts.tile([128, MBN], F32, tag="wob8")
            esk = consts.tile([128, HL], F32, tag="esk")
            ident_f = consts.tile([128, 128], F32, tag="ident_f")
            ident = consts.tile([128, 128], BF16, tag="ident")
            ones = consts.tile([128, 1], F32, tag="ones")
            for t, src_ in [(bqt, bq), (bkt, bk), (bvt, bv), (cost, cosT),
                            (sint, sinTs), (wob8t, wob8), (esk, esink)]:
                nc.sync.dma_start(out=t[:, :], in_=src_[:, :])
            make_identity(nc, ident_f[:, :])
            nc.vector.tensor_copy(ident[:, :], ident_f[:, :])
            nc.vector.memset(ones[:, :], 1.0)
            tri = consts.tile([128, 128], BF16, tag="tri")
            nc.vector.memset(tri[:, :], 1.0)
            nc.gpsimd.affine_select(
                out=tri[:, :], in_=tri[:, :],
                compare_op=mybir.AluOpType.is_ge,
                fill=0.0, base=0, pattern=[[1, 128]], channel_multiplier=-1,
            )

            qq = qkvout.tile([64, HL * S], BF16, tag="qq")
            kh = qkvout.tile([64, S], BF16, tag="kh")
            vT = qkvout.tile([64, S], BF16, tag="vT")
            vaug = qkvout.tile([128, KBN * VA], BF16, tag="vaug")

            # ---------------- QKV projection (piece-streamed) ----------------
            with (
                tc.tile_pool(name="wtp", bufs=1) as wtp,
                tc.tile_pool(name="xcp", bufs=3) as xcp,
                tc.tile_pool(name="qkps", bufs=4, space="PSUM") as qkps,
                tc.tile_pool(name="rtmp", bufs=3) as rtmp,
            ):
                wts = []
                for pc, (cb0, ncb) in enumerate(PIECES):
                    wt_pc = wtp.tile([128, ncb * 640], BF16, tag=f"wt{pc}",
                                     name=f"wt{pc}")
                    nc.sync.dma_start(
                        out=wt_pc[:, :].rearrange("p (cb n) -> p cb n", cb=ncb),
                        in_=wT[cb0 * 128 : (cb0 + ncb) * 128, :].rearrange(
                            "(cb p) n -> p cb n", p=128
                        ),
                    )
                    wts.append(wt_pc)
                wt2 = wtp.tile([64, 640], BF16, tag="wtail")
                nc.sync.dma_start(out=wt2[:, :], in_=wT[CBF * 128 : HID, :])

                for sc in range(NSC):
                    c0 = sc * SCQ
                    xps = []
                    for pc, (cb0, ncb) in enumerate(PIECES):
                        xp = xcp.tile([128, ncb * SCQ], BF16, tag=f"xp{pc}",
                                      name=f"xp{pc}_{sc}")
                        nc.sync.dma_start(
                            out=xp[:, :].rearrange("p (cb s) -> p cb s", cb=ncb),
                            in_=xT[cb0 * 128 : (cb0 + ncb) * 128, c0 : c0 + SCQ].rearrange(
                                "(cb p) s -> p cb s", p=128
                            ),
                        )
                        xps.append(xp)
                    xc2 = xcp.tile([64, SCQ], BF16, tag="xc2", name=f"xc2_{sc}")
                    nc.sync.dma_start(
                        out=xc2[:, :], in_=xT[CBF * 128 : HID, c0 : c0 + SCQ]
                    )


                    for nb in range(5):
                        p = qkps.tile([128, SCQ], F32, tag="qkv", name=f"qk{nb}_{sc}")
                        for pc, (cb0, ncb) in enumerate(PIECES):
                            for i in range(ncb):
                                nc.tensor.matmul(
                                    p[:, :],
                                    wts[pc][:, i * 640 + nb * 128 : i * 640 + (nb + 1) * 128],
                                    xps[pc][:, i * SCQ : (i + 1) * SCQ],
                                    start=(pc == 0 and i == 0),
                                    stop=False,
                                )
                        nc.tensor.matmul(
                            p[:, :],
                            wt2[:, nb * 128 : (nb + 1) * 128],
                            xc2[:, :],
                            start=False,
                            stop=True,
                        )
                        if nb < 4:
                            for half in range(2):
                                h = 2 * nb + half
                                hb = 64 * half
                                qb = rtmp.tile([64, SCQ], F32, tag="qb",
                                               name=f"qb_{sc}_{nb}_{half}")
                                nc.vector.tensor_scalar_add(
                                    qb[:, :], p[hb : hb + 64, :], bqt[:, h : h + 1]
                                )
                                rot = rtmp.tile([64, SCQ], F32, tag="rot",
                                                name=f"rot_{sc}_{nb}_{half}")
                                nc.scalar.copy(rot[0:32, :], qb[32:64, :])
                                nc.scalar.copy(rot[32:64, :], qb[0:32, :])
                                nc.vector.tensor_mul(
                                    qb[:, :], qb[:, :], cost[:, c0 : c0 + SCQ]
                                )
                                nc.vector.tensor_mul(
                                    rot[:, :], rot[:, :], sint[:, c0 : c0 + SCQ]
                                )
                                nc.vector.tensor_add(
                                    qq[:, h * S + c0 : h * S + c0 + SCQ],
                                    qb[:, :],
                                    rot[:, :],
                                )
                        else:
                            kb_ = rtmp.tile([64, SCQ], F32, tag="qb",
                                            name=f"kb_{sc}")
                            nc.vector.tensor_scalar_add(
                                kb_[:, :], p[0:64, :], bkt[:, 0:1]
                            )
                            rot = rtmp.tile([64, SCQ], F32, tag="rot",
                                            name=f"krot_{sc}")
                            nc.scalar.copy(rot[0:32, :], kb_[32:64, :])
                            nc.scalar.copy(rot[32:64, :], kb_[0:32, :])
                            nc.vector.tensor_mul(
                                kb_[:, :], kb_[:, :], cost[:, c0 : c0 + SCQ]
                            )
                            nc.vector.tensor_mul(
                                rot[:, :], rot[:, :], sint[:, c0 : c0 + SCQ]
                            )
                            nc.vector.tensor_add(
                                kh[:, c0 : c0 + SCQ], kb_[:, :], rot[:, :]
                            )
                            nc.vector.tensor_scalar_add(
                                vT[:, c0 : c0 + SCQ], p[64:128, :], bvt[:, 0:1]
                            )

            # ---------------- v transpose + ones column ----------------
            with tc.tile_pool(name="vtp", bufs=2, space="PSUM") as vtp:
                for kb in range(KBN):
                    pv = vtp.tile([128, D], BF16, tag="pv", name=f"pv{kb}")
                    nc.tensor.transpose(
                        pv[:, :], vT[:, kb * 128 : (kb + 1) * 128], ident[0:64, 0:64]
                    )
                    nc.vector.tensor_copy(vaug[:, kb * VA : kb * VA + 64], pv[:, :])
                    for oc in range(64, VA):
                        nc.vector.tensor_copy(
                            vaug[:, kb * VA + oc : kb * VA + oc + 1], ones[:, :]
                        )

            # ---------------- attention + wo + chunked ReduceScatter ----------------
            with (
                tc.tile_pool(name="oTp", bufs=1) as oTp,
                tc.tile_pool(name="woTp", bufs=1) as woTp,
                tc.tile_pool(name="esp", bufs=8) as esp,
                tc.tile_pool(name="scps", bufs=2, space="PSUM") as scps,
                tc.tile_pool(name="pops", bufs=1, space="PSUM") as pops,
                tc.tile_pool(name="wops", bufs=2, space="PSUM") as wops,
            ):
                oTt = oTp.tile([128, JBN * S], BF16, tag="oT")
                woTt = woTp.tile([128, JBN * HID], BF16, tag="woT")
                nc.sync.dma_start(
                    out=woTt[:, :].rearrange("p (jb m) -> p jb m", jb=JBN),
                    in_=woT[:, :].rearrange("(jb p) m -> p jb m", p=128),
                )

                for qc in QC_ORDER:
                    q0 = qc * QC
                    qb0 = q0 // 128
                    for h in range(HL):
                        nkb = qb0 + 4
                        es_tiles = [None] * nkb
                        po = [
                            pops.tile([128, VA], F32, tag=f"po{s}", name=f"po{s}_{qc}_{h}")
                            for s in range(4)
                        ]

                        def emit_scores(kb):
                            j = kb - qb0
                            w0 = 128 * j if j > 0 else 0
                            w = QC - w0
                            ps_s = scps.tile([128, QC], F32, tag="scores",
                                             name=f"ps_{qc}_{h}_{kb}")
                            nc.tensor.matmul(
                                ps_s[:, 0:w],
                                kh[:, kb * 128 : (kb + 1) * 128],
                                qq[:, h * S + q0 + w0 : h * S + q0 + QC],
                                start=True,
                                stop=True,
                            )
                            es = esp.tile([128, QC], BF16, tag="es",
                                          name=f"es_{qc}_{h}_{kb}")
                            nc.scalar.activation(
                                es[:, 0:w], ps_s[:, 0:w], Exp, scale=0.125
                            )
                            if j >= 0:
                                if kb % 2 == 0:
                                    nc.gpsimd.affine_select(
                                        out=es[:, 0:128],
                                        in_=es[:, 0:128],
                                        compare_op=mybir.AluOpType.is_ge,
                                        fill=0.0,
                                        base=0,
                                        pattern=[[1, 128]],
                                        channel_multiplier=-1,
                                    )
                                else:
                                    nc.vector.tensor_mul(
                                        es[:, 0:128], es[:, 0:128], tri[:, :]
                                    )
                            es_tiles[kb] = es

                        def emit_avs(kb):
                            j = kb - qb0
                            for s in range(4):
                                if qb0 + s >= kb:
                                    c0_ = (s - j) * 128 if j > 0 else s * 128
                                    nc.tensor.matmul(
                                        po[s][:, :],
                                        es_tiles[kb][:, c0_ : c0_ + 128],
                                        vaug[:, kb * VA : (kb + 1) * VA],
                                        start=(kb == 0),
                                        stop=(kb == qb0 + s),
                                    )

                        for kb in range(nkb):
                            emit_scores(kb)
                            if kb >= 2:
                                emit_avs(kb - 2)
                        emit_avs(nkb - 2)
                        emit_avs(nkb - 1)

                        for s in range(4):
                            denom = small.tile([128, 1], F32, tag="denom",
                                               name=f"dn_{qc}_{h}_{s}")
                            nc.vector.tensor_add(
                                denom[:, :], po[s][:, D : D + 1], esk[:, h : h + 1]
                            )
                            recip = small.tile([128, 1], F32, tag="recip",
                                               name=f"rc_{qc}_{h}_{s}")
                            nc.vector.reciprocal(recip[:, :], denom[:, :])
                            o_n = small.tile([128, D], BF16, tag="o_n",
                                             name=f"on_{qc}_{h}_{s}")
                            nc.vector.tensor_scalar_mul(
                                o_n[:, :], po[s][:, 0:D], recip[:, :]
                            )
                            pt = pops.tile([64, 128], BF16, tag=f"po{s}",
                                           name=f"pt_{qc}_{h}_{s}")
                            nc.tensor.transpose(pt[:, :], o_n[:, :], ident[:, :])
                            jb, ro = h // 2, (h % 2) * 64
                            dst = oTt[
                                ro : ro + 64,
                                jb * S + q0 + s * 128 : jb * S + q0 + (s + 1) * 128,
                            ]
                            if s % 2 == 0:
                                nc.vector.tensor_copy(dst, pt[:, :])
                            else:
                                nc.scalar.copy(dst, pt[:, :])

                    tc.strict_bb_all_engine_barrier()
                    # wo + RS for this chunk (possibly split into column segments)
                    for off, w in WO_SEGS[qc]:
                        for mb in range(MBN):
                            rows = 128 if mb < CBF else 64
                            pw = wops.tile([128, QC], F32, tag="wo",
                                           name=f"pw_{qc}_{off}_{mb}")
                            for jb in range(JBN):
                                nc.tensor.matmul(
                                    pw[0:rows, 0:w],
                                    woTt[:, jb * HID + mb * 128 : jb * HID + mb * 128 + rows],
                                    oTt[:, jb * S + q0 + off : jb * S + q0 + off + w],
                                    start=(jb == 0),
                                    stop=(jb == JBN - 1),
                                )
                            yt = ytp.tile([128, QC], BF16, tag="ytb",
                                          name=f"yt_{qc}_{off}_{mb}")
                            if mb % 2 == 0:
                                nc.vector.tensor_scalar_add(
                                    yt[0:rows, 0:w], pw[0:rows, 0:w],
                                    wob8t[0:rows, mb : mb + 1],
                                )
                            else:
                                nc.scalar.activation(
                                    yt[0:rows, 0:w], pw[0:rows, 0:w],
                                    mybir.ActivationFunctionType.Identity,
                                    bias=wob8t[0:rows, mb : mb + 1],
                                )
                            nc.sync.dma_start(
                                out=yT_part[(qc, off)][mb * 128 : mb * 128 + rows, :],
                                in_=yt[0:rows, 0:w],
                            )
                            if qc == 0 and mb == 10:
                                nc.gpsimd.collective_compute(
                                    "ReduceScatter",
                                    mybir.AluOpType.add,
                                    replica_groups=[list(range(CORES))],
                                    ins=[yT_part[(qc, off)][0:1408, :].opt()],
                                    outs=[yT_red[(qc, off)][0:176, :].opt()],
                                )
                        if qc == 0:
                            nc.gpsimd.collective_compute(
                                "ReduceScatter",
                                mybir.AluOpType.add,
                                replica_groups=[list(range(CORES))],
                                ins=[yT_part[(qc, off)][1408:HID, :].opt()],
                                outs=[yT_red[(qc, off)][176:360, :].opt()],
                            )
                        else:
                            nc.gpsimd.collective_compute(
                                "ReduceScatter",
                                mybir.AluOpType.add,
                                replica_groups=[list(range(CORES))],
                                ins=[yT_part[(qc, off)].ap().opt()],
                                outs=[yT_red[(qc, off)].ap().opt()],
                            )

            # ---------------- bias tail ----------------
            for t in range(3):
                rows = 128 if t < 2 else 104
                for qc in range(NQC):
                    for off, w in WO_SEGS[qc]:
                        rt = ytp.tile([128, QC], BF16, tag="ytb",
                                      name=f"rt_{t}_{qc}_{off}")
                        nc.sync.dma_start(
                            out=rt[0:rows, 0:w],
                            in_=yT_red[(qc, off)][t * 128 : t * 128 + rows, :],
                        )
                        rt32 = ytp.tile([128, QC], F32, tag="yt32",
                                        name=f"rt32_{t}_{qc}_{off}")
                        nc.vector.tensor_copy(rt32[0:rows, 0:w], rt[0:rows, 0:w])
                        nc.sync.dma_start(
                            out=out[t * 128 : t * 128 + rows,
                                    qc * QC + off : qc * QC + off + w],
                            in_=rt32[0:rows, 0:w],
                        )

    nc.finalize()
    return nc


def make_in_maps(x, rope_cache, wq_w, wq_b, wk_w, wk_b, wv_w, wv_b, wo_w, wo_b, sinks):
    import ml_dtypes

    BF = ml_dtypes.bfloat16
    xT = np.ascontiguousarray(x[0].T).astype(BF)  # [2880, 1536]
    cosT = np.ascontiguousarray(rope_cache[:, :D].T, dtype=np.float32)
    sinT = np.ascontiguousarray(rope_cache[:, D:].T, dtype=np.float32)
    sinTs = sinT.copy()
    sinTs[: D // 2] *= -1.0

    in_maps = []
    for i in range(CORES):
        wq = wq_w[512 * i : 512 * (i + 1)]
        wk = wk_w[64 * i : 64 * (i + 1)]
        wv = wv_w[64 * i : 64 * (i + 1)]
        wT = np.ascontiguousarray(np.concatenate([wq, wk, wv], axis=0).T).astype(BF)
        bq = np.ascontiguousarray(
            wq_b[512 * i : 512 * (i + 1)].reshape(HL, 64).T, np.float32
        )
        bk = wk_b[64 * i : 64 * (i + 1)].reshape(64, 1).astype(np.float32)
        bv = wv_b[64 * i : 64 * (i + 1)].reshape(64, 1).astype(np.float32)
        woT = np.ascontiguousarray(wo_w[:, 512 * i : 512 * (i + 1)].T).astype(BF)
        wob8 = np.zeros((128, MBN), np.float32)
        for mb in range(MBN):
            piece = wo_b[128 * mb : 128 * (mb + 1)] / 8.0
            wob8[: len(piece), mb] = piece
        es = np.exp(sinks[HL * i : HL * (i + 1)]).astype(np.float32)
        esink = np.repeat(es[None, :], 128, axis=0).astype(np.float32)
        in_maps.append(
            {
                "xT": xT,
                "wT": wT,
                "bq": bq,
                "bk": bk,
                "bv": bv,
                "cosT": cosT,
                "sinTs": sinTs,
                "woT": woT,
                "wob8": np.ascontiguousarray(wob8),
                "esink": esink,
            }
        )
    return in_maps


_CACHE = {}


def kernel(**inputs):
    _install_hooks()
    from concourse import bass_utils

    trace = bool(int(os.environ.get("BASS_KERNEL_TRACE", "0")))
    if "nc" not in _CACHE:
        _CACHE["nc"] = build_graph()
    nc = _CACHE["nc"]

    in_maps = make_in_maps(**{k: np.asarray(v) for k, v in inputs.items()})
    res = bass_utils.run_bass_kernel_spmd(
        nc, in_maps, core_ids=list(range(CORES)), trace=trace
    )
    _EXEC_TIME_NS[0] = res.exec_time_ns

    y = np.empty((S, HID), np.float32)
    for i in range(CORES):
        o = res.results[i]["out"]
        y[512:, 360 * i : 360 * (i + 1)] = o[:, 512:].T
        y[0:512, 176 * i : 176 * (i + 1)] = o[0:176, 0:512].T
        y[0:512, 1408 + 184 * i : 1408 + 184 * (i + 1)] = o[176:360, 0:512].T
    return y.reshape(1, S, HID)


def last_exec_time_ns():
    return _EXEC_TIME_NS[0]
